# revision 1
# baseline (speedup 1.0000x reference)
"""Trainium2 Bass kernel for nn_MultiHeadAttention_16509854286463.

Multi-head attention (B=4, N=2048, D=1024, H=16, HD=64, RD=32) with
interleaved partial RoPE, causal mask, all-zero pad mask/biases.

Sharding: 8 cores = 4 batches x 2 head-groups (8 heads each).
Each core computes q/k/v projections for its head-group on its batch,
attention, and a row-parallel slice of the output projection; the host
sums the two partial o_proj results per batch (tensor-parallel reduce)
and adds the output bias.

Device dataflow (per core):
  phase 1 (per 512-token s-chunk): xT tiles -> Q^T,K^T (hd-on-partition
    layout, f32r) with RoPE applied via a constant signed-permutation
    matmul (rotate_half) + cos/sin elementwise ops; V in (seq, hd)
    layout with a ones column appended for softmax sums.
  phase 2 (per head-pair, per 512-query chunk): S^T = K^T.T @ Q^T per
    128-key block (keys on psum partitions, queries on free dim),
    causal triangle mask added on diagonal blocks, exp on ScalarE with
    the 1/sqrt(HD) scale folded in, then O'^T = [V|1].T @ expS
    accumulated over key blocks (row 64 = softmax denominators).
    Normalization multiplies by a K=1-matmul broadcast of 1/sums.
  phase 3: y^T = Wo_g.T @ O^T (row-parallel o_proj partial).
"""

import numpy as np
import ml_dtypes

B, N, D = 4, 2048, 1024
H, HD, RD = 16, 64, 32
HG = 8            # heads per core (head-group)
JG = HG * HD      # 512 j-dims per core
SC = 512          # s-chunk
NSC = N // SC     # 4 s-chunks
NP = 4            # head pairs per core
KB = 128          # key block
NKB = N // KB     # 16 key blocks
KT8 = D // 128    # 8 contraction tiles for projections
NEG = -3.0e5      # additive causal mask (pre exp-scale)

_CACHE = {}


def _build_nc():
    import concourse.bass as bass
    import concourse.mybir as mybir
    import concourse.tile as tile
    from concourse import bacc
    from contextlib import ExitStack

    F32 = mybir.dt.float32
    F32R = mybir.dt.float32r
    BF16 = mybir.dt.bfloat16
    EXP = mybir.ActivationFunctionType.Exp

    nc = bacc.Bacc()

    xq_d = nc.dram_tensor("xqT", [D, N], F32R, kind="ExternalInput")
    xk_d = nc.dram_tensor("xkT", [D, N], F32R, kind="ExternalInput")
    wq_d = nc.dram_tensor("wq", [D, JG], F32R, kind="ExternalInput")
    wk_d = nc.dram_tensor("wk", [D, JG], F32R, kind="ExternalInput")
    wv_d = nc.dram_tensor("wv", [D, JG], F32R, kind="ExternalInput")
    wo_d = nc.dram_tensor("wo", [JG, D], BF16, kind="ExternalInput")
    cos_d = nc.dram_tensor("cosE", [128, N], BF16, kind="ExternalInput")
    sin_d = nc.dram_tensor("sinE", [128, N], BF16, kind="ExternalInput")
    rm_d = nc.dram_tensor("rmat", [128, 128], F32R, kind="ExternalInput")
    tm_d = nc.dram_tensor("trimask", [128, 128], F32, kind="ExternalInput")
    y_d = nc.dram_tensor("yT", [D, N], F32, kind="ExternalOutput")

    xq_t = xq_d.ap().rearrange("(o p) s -> p o s", p=128)
    xk_t = xk_d.ap().rearrange("(o p) s -> p o s", p=128)
    wq_t = wq_d.ap().rearrange("(o p) j -> p o j", p=128)
    wk_t = wk_d.ap().rearrange("(o p) j -> p o j", p=128)
    wv_t = wv_d.ap().rearrange("(o p) j -> p o j", p=128)
    wo_t = wo_d.ap().rearrange("(o p) d -> p o d", p=128)

    with tile.TileContext(nc) as tc, ExitStack() as ctx:
        consts = ctx.enter_context(tc.tile_pool(name="consts", bufs=1))
        persist = ctx.enter_context(tc.tile_pool(name="persist", bufs=1))
        qt_pool = ctx.enter_context(tc.tile_pool(name="qt", bufs=2))
        y_pool = ctx.enter_context(tc.tile_pool(name="ysb", bufs=4))
        ictx = ctx.enter_context(ExitStack())
        x_pool = ictx.enter_context(tc.tile_pool(name="x", bufs=1))
        tmp_pool = ictx.enter_context(tc.tile_pool(name="tmp", bufs=2))
        es_pool = ictx.enter_context(tc.tile_pool(name="es", bufs=4))
        nr_pool = ictx.enter_context(tc.tile_pool(name="nr", bufs=2))
        ps_gen = ictx.enter_context(tc.tile_pool(name="psgen", bufs=2, space="PSUM"))
        ps_st = ictx.enter_context(tc.tile_pool(name="psst", bufs=2, space="PSUM"))
        ps_ov = ictx.enter_context(tc.tile_pool(name="psov", bufs=1, space="PSUM"))
        dr_pool = ictx.enter_context(tc.tile_pool(name="dr", bufs=4, space="DRAM"))

        # ---- constants; order matters: first matmuls need wv + x(sc=0) ----
        wv_sb = consts.tile([128, KT8, JG], F32R, tag="wv")
        wq_sb = consts.tile([128, KT8, JG], F32R, tag="wq")
        wk_sb = consts.tile([128, KT8, JG], F32R, tag="wk")
        x0 = {}
        for nm in ("xk", "xq"):
            x0[nm] = x_pool.tile([128, KT8, SC], F32R, tag=nm, name=nm + "0")
        for k in range(KT8):
            nc.sync.dma_start(out=x0["xk"][:, k, :], in_=xk_t[:, k, 0:SC])
            nc.sync.dma_start(out=wv_sb[:, k, :], in_=wv_t[:, k, :])
            nc.sync.dma_start(out=x0["xq"][:, k, :], in_=xq_t[:, k, 0:SC])
        rmat = consts.tile([128, 128], F32R, tag="rmat")
        nc.sync.dma_start(out=rmat[:, :], in_=rm_d[:, :])
        cosE = consts.tile([128, N], BF16, tag="cosE")
        sinE = consts.tile([128, N], BF16, tag="sinE")
        nc.sync.dma_start(out=cosE[:, :], in_=cos_d[:, :])
        nc.sync.dma_start(out=sinE[:, :], in_=sin_d[:, :])
        trimask = consts.tile([128, 128], F32, tag="trimask")
        nc.sync.dma_start(out=trimask[:, :], in_=tm_d[:, :])
        wo_sb = consts.tile([128, 4, D], BF16, tag="wo")

        # persistent activations
        KTt = [[persist.tile([128, SC], BF16, tag=f"kt_{p}_{s}", name=f"kt_{p}_{s}")
                for s in range(NSC)] for p in range(NP)]
        Vt = [persist.tile([128, HG, HD + 1], BF16, tag=f"v_{i}", name=f"v_{i}")
              for i in range(NKB)]
        OTt = [[persist.tile([128, SC], BF16, tag=f"ot_{p}_{q}", name=f"ot_{p}_{q}")
                for q in range(NSC)] for p in range(NP)]

        def attention(p, qc):
            h0, h1 = 2 * p, 2 * p + 1
            nkb = 4 * qc + 4
            ov = [ps_ov.tile([65, SC], F32, tag=f"ov{i}", name=f"ov{i}") for i in range(2)]
            qt = QTt[p]
            for kb in range(nkb):
                diag = kb >= 4 * qc
                m = kb - 4 * qc
                skt = KTt[p][kb // 4]
                lo = (kb % 4) * KB
                st = ps_st.tile([128, 2 * SC], F32, tag="st")
                es = es_pool.tile([128, 2 * SC], BF16, tag="es")
                for hl in (0, 1):
                    r0, r1 = hl * 64, hl * 64 + 64
                    base = hl * SC
                    c0 = m * KB if diag else 0
                    # f32r runs 4 cy/row below N=256; widen the last diag
                    # block's matmul (extra cols are never exp'd/read)
                    cm = min(c0, SC - 256)
                    nc.tensor.matmul(
                        st[:, base + cm:base + SC],
                        skt[r0:r1, lo:lo + KB],
                        qt[r0:r1, cm:SC],
                        start=True, stop=True)
                    if diag:
                        nc.vector.tensor_add(
                            out=st[:, base + c0:base + c0 + KB],
                            in0=st[:, base + c0:base + c0 + KB],
                            in1=trimask[:, :])
                        if m > 0:
                            nc.gpsimd.memset(es[:, base:base + c0], 0.0)
                        nc.scalar.activation(
                            out=es[:, base + c0:base + SC],
                            in_=st[:, base + c0:base + SC],
                            func=EXP, scale=float(HD) ** -0.5)
                if not diag:
                    # one wide exp across both heads' score halves
                    nc.scalar.activation(
                        out=es[:, :], in_=st[:, :],
                        func=EXP, scale=float(HD) ** -0.5)
                for hl, h in ((0, h0), (1, h1)):
                    nc.tensor.matmul(
                        ov[hl][:, :],
                        Vt[kb][:, h, :],
                        es[:, hl * SC:hl * SC + SC],
                        start=(kb == 0), stop=(kb == nkb - 1))
            for hl in (0, 1):
                ovs = nr_pool.tile([65, SC], F32, tag="ovs")
                nc.vector.tensor_copy(out=ovs[:, :], in_=ov[hl][:, :])
                rc = nr_pool.tile([65, SC], F32, tag="rc")
                nc.vector.reciprocal(out=rc[64:65, :], in_=ovs[64:65, :])
                scr = dr_pool.tile([1, SC], F32, tag="scr", name="scr")
                nc.sync.dma_start(out=scr[:, :], in_=rc[64:65, :])
                rb = nr_pool.tile([64, SC], F32, tag="rb")
                nc.sync.dma_start(out=rb[:, :],
                                  in_=scr[0:1, :].partition_broadcast(64))
                if hl == 0:
                    nc.vector.tensor_mul(out=OTt[p][qc][0:64, :],
                                         in0=ovs[0:64, :], in1=rb[:, :])
                else:
                    nr = nr_pool.tile([64, SC], BF16, tag="nr")
                    nc.vector.tensor_mul(out=nr[:, :], in0=ovs[0:64, :],
                                         in1=rb[:, :])
                    nc.sync.dma_start(out=OTt[p][qc][64:128, :],
                                      in_=nr[:, :])

        def oproj_chunk(qc, dcs=None):
            for dc in (range(KT8) if dcs is None else dcs):
                yp = ps_gen.tile([128, SC], F32, tag="gen", name="yp")
                for kt in range(4):
                    nc.tensor.matmul(
                        yp[:, :],
                        wo_sb[:, kt, dc * 128:(dc + 1) * 128],
                        OTt[kt][qc][:, :],
                        start=(kt == 0), stop=(kt == 3))
                ysb = y_pool.tile([128, SC], F32, tag="ysb", name="ysb")
                if dc % 2 == 0:
                    nc.vector.tensor_copy(out=ysb[:, :], in_=yp[:, :])
                else:
                    nc.scalar.copy(out=ysb[:, :], in_=yp[:, :])
                nc.sync.dma_start(
                    out=y_d[dc * 128:(dc + 1) * 128, qc * SC:(qc + 1) * SC],
                    in_=ysb[:, :])

        for sc in range(NSC):
            # ---- phase 1: x loads, V projection, Q/K projection + RoPE ----
            if sc == 0:
                xq_sb, xk_sb = x0["xq"], x0["xk"]
            else:
                xq_sb = x_pool.tile([128, KT8, SC], F32R, tag="xq", name="xq")
                xk_sb = x_pool.tile([128, KT8, SC], F32R, tag="xk", name="xk")
                for k in range(KT8):
                    nc.sync.dma_start(out=xq_sb[:, k, :],
                                      in_=xq_t[:, k, sc * SC:(sc + 1) * SC])
                    nc.sync.dma_start(out=xk_sb[:, k, :],
                                      in_=xk_t[:, k, sc * SC:(sc + 1) * SC])

            # V projection: per 128-seq subtile
            for ss in range(4):
                sidx = sc * 4 + ss
                vp = ps_gen.tile([128, SC], F32, tag="gen", name="vp")
                for k in range(KT8):
                    nc.tensor.matmul(
                        vp[:, :],
                        xk_sb[:, k, ss * 128:(ss + 1) * 128],
                        wv_sb[:, k, :],
                        start=(k == 0), stop=(k == KT8 - 1))
                vt = Vt[sidx]
                nc.any.tensor_copy(
                    out=vt[:, :, 0:HD],
                    in_=vp[:, :].rearrange("p (h d) -> p h d", h=HG))
                nc.vector.memset(vt[:, :, HD:HD + 1], 1.0)

            # Q/K projections + RoPE per head pair
            QTt = [None] * NP
            for p in range(NP):
                QTt[p] = qt_pool.tile([128, SC], BF16, tag=f"qt_{p}", name=f"qt_{p}")
            for t, (x_sb, w_sb, w_t) in enumerate(
                    ((xq_sb, wq_sb, wq_t), (xk_sb, wk_sb, wk_t))):
                for p in range(NP):
                    if sc == 0:
                        nc.sync.dma_start(
                            out=w_sb[:, :, p * 128:(p + 1) * 128],
                            in_=w_t[:, :, p * 128:(p + 1) * 128])
                    pp = ps_gen.tile([128, SC], F32, tag="gen", name="pp")
                    for k in range(KT8):
                        nc.tensor.matmul(pp[:, :],
                                         w_sb[:, k, p * 128:(p + 1) * 128],
                                         x_sb[:, k, :],
                                         start=(k == 0), stop=(k == KT8 - 1))
                    raw = tmp_pool.tile([128, SC], F32R, tag="raw")
                    nc.any.tensor_copy(out=raw[:, :], in_=pp[:, :])
                    rp = ps_gen.tile([128, SC], F32, tag="gen", name="rp")
                    nc.tensor.matmul(rp[:, :], rmat[:, :], raw[:, :],
                                     start=True, stop=True)
                    dest = QTt[p] if t == 0 else KTt[p][sc]
                    cs = slice(sc * SC, (sc + 1) * SC)
                    nc.vector.tensor_mul(out=dest[:, :], in0=raw[:, :],
                                         in1=cosE[:, cs])
                    tsin = tmp_pool.tile([128, SC], F32, tag="tsin")
                    nc.vector.tensor_mul(out=tsin[:, :], in0=rp[:, :],
                                         in1=sinE[:, cs])
                    nc.vector.tensor_add(out=dest[:, :], in0=dest[:, :],
                                         in1=tsin[:, :])

            if sc == 1:
                nc.sync.dma_start(out=wo_sb[:, :, :], in_=wo_t[:, :, :])
            # ---- phase 2: attention for q-chunk sc, all pairs, woven with
            # the previous q-chunk's o_proj (fills PE bubbles during the
            # ACT-bound attention stretch) ----
            for p in range(NP):
                attention(p, sc)
                if sc > 0:
                    oproj_chunk(sc - 1, range(2 * p, 2 * p + 2))

        # ---- final phase 3 chunk with fresh deep pools ----
        ictx.close()
        ps_y = ctx.enter_context(tc.tile_pool(name="psy", bufs=5, space="PSUM"))
        for dc in range(KT8):
            qc = NSC - 1
            yp = ps_y.tile([128, SC], F32, tag="yp", name="yp")
            for kt in range(4):
                nc.tensor.matmul(
                    yp[:, :],
                    wo_sb[:, kt, dc * 128:(dc + 1) * 128],
                    OTt[kt][qc][:, :],
                    start=(kt == 0), stop=(kt == 3))
            ysb = y_pool.tile([128, SC], F32, tag="ysb", name="ysb")
            if dc % 2 == 0:
                nc.vector.tensor_copy(out=ysb[:, :], in_=yp[:, :])
            else:
                nc.scalar.copy(out=ysb[:, :], in_=yp[:, :])
            nc.sync.dma_start(
                out=y_d[dc * 128:(dc + 1) * 128, qc * SC:(qc + 1) * SC],
                in_=ysb[:, :])

    nc.compile()
    return nc


def _host_consts(pos_enc):
    pe = np.asarray(pos_enc, np.float32)[0]          # (N, RD)
    cos = np.cos(pe).T                               # (RD, N)
    sin = np.sin(pe).T
    blk_c = np.ones((HD, N), np.float32)
    blk_c[:RD] = cos
    blk_s = np.zeros((HD, N), np.float32)
    blk_s[:RD] = sin
    cosE = np.tile(blk_c, (2, 1))                    # (128, N)
    sinE = np.tile(blk_s, (2, 1))
    rmat = np.zeros((128, 128), np.float32)
    for o in (0, HD):
        for i in range(RD // 2):
            rmat[o + 2 * i + 1, o + 2 * i] = -1.0
            rmat[o + 2 * i, o + 2 * i + 1] = 1.0
    r = np.arange(128)[:, None]
    c = np.arange(128)[None, :]
    trimask = np.where(c >= r, 0.0, NEG).astype(np.float32)
    return cosE, sinE, rmat, trimask


def kernel(x_q, x_kv, pos_enc, Wq, bq, Wk, bk, Wv, bv, Wo, bo, pad_mask):
    from concourse.bass_utils import run_bass_kernel_spmd

    if "nc" not in _CACHE:
        _CACHE["nc"] = _build_nc()
    nc = _CACHE["nc"]

    x_q = np.asarray(x_q, np.float32)
    x_kv = np.asarray(x_kv, np.float32)
    Wq = np.asarray(Wq, np.float32)
    Wk = np.asarray(Wk, np.float32)
    Wv = np.asarray(Wv, np.float32)
    Wo = np.asarray(Wo, np.float32)
    bo = np.asarray(bo, np.float32)

    cosE, sinE, rmat, trimask = _host_consts(pos_enc)

    in_maps = []
    for core in range(8):
        b, g = core // 2, core % 2
        js = slice(g * JG, (g + 1) * JG)
        in_maps.append({
            "xqT": np.ascontiguousarray(x_q[b].T),
            "xkT": np.ascontiguousarray(x_kv[b].T),
            "wq": np.ascontiguousarray(Wq[:, js]),
            "wk": np.ascontiguousarray(Wk[:, js]),
            "wv": np.ascontiguousarray(Wv[:, js]),
            "wo": np.ascontiguousarray(Wo[js, :]).astype(ml_dtypes.bfloat16),
            "cosE": cosE.astype(ml_dtypes.bfloat16), "sinE": sinE.astype(ml_dtypes.bfloat16),
            "rmat": rmat, "trimask": trimask,
        })

    res = run_bass_kernel_spmd(nc, in_maps, list(range(8)))

    out = np.empty((B, N, D), np.float32)
    for b in range(B):
        out[b] = res.results[2 * b]["yT"].T + res.results[2 * b + 1]["yT"].T
    out += bo
    return out



# revision 28
# speedup vs baseline: 1.1692x; 1.1692x over previous
"""Trainium2 Bass kernel for nn_MultiHeadAttention_16509854286463.

Multi-head attention (B=4, N=2048, D=1024, H=16, HD=64, RD=32) with
interleaved partial RoPE, causal mask, all-zero pad mask/biases.

Sharding: 8 cores = 4 batches x 2 head-groups (8 heads each).
Each core computes q/k/v projections for its head-group on its batch,
attention, and a row-parallel slice of the output projection; the host
sums the two partial o_proj results per batch (tensor-parallel reduce)
and adds the output bias.

Device dataflow (per core):
  phase 1 (per 512-token s-chunk): xT tiles -> Q^T,K^T (hd-on-partition
    layout, f32r) with RoPE applied via a constant signed-permutation
    matmul (rotate_half) + cos/sin elementwise ops; V in (seq, hd)
    layout with a ones column appended for softmax sums.
  phase 2 (per head-pair, per 512-query chunk): S^T = K^T.T @ Q^T per
    128-key block (keys on psum partitions, queries on free dim),
    causal triangle mask added on diagonal blocks, exp on ScalarE with
    the 1/sqrt(HD) scale folded in, then O'^T = [V|1].T @ expS
    accumulated over key blocks (row 64 = softmax denominators).
    Normalization multiplies by a K=1-matmul broadcast of 1/sums.
  phase 3: y^T = Wo_g.T @ O^T (row-parallel o_proj partial).
"""

import numpy as np
import ml_dtypes

B, N, D = 4, 2048, 1024
H, HD, RD = 16, 64, 32
HG = 8            # heads per core (head-group)
JG = HG * HD      # 512 j-dims per core
SC = 512          # s-chunk
NSC = N // SC     # 4 s-chunks
NP = 4            # head pairs per core
KB = 128          # key block
NKB = N // KB     # 16 key blocks
KT8 = D // 128    # 8 contraction tiles for projections
NEG = -3.0e5      # additive causal mask (pre exp-scale)

_CACHE = {}


def _build_nc():
    import concourse.bass as bass
    import concourse.mybir as mybir
    import concourse.tile as tile
    from concourse import bacc
    from contextlib import ExitStack

    F32 = mybir.dt.float32
    F32R = mybir.dt.float32r
    BF16 = mybir.dt.bfloat16
    EXP = mybir.ActivationFunctionType.Exp

    nc = bacc.Bacc()

    xq_d = nc.dram_tensor("xqT", [D, N], BF16, kind="ExternalInput")
    xk_d = nc.dram_tensor("xkT", [D, N], BF16, kind="ExternalInput")
    wq_d = nc.dram_tensor("wq", [D, JG], BF16, kind="ExternalInput")
    wk_d = nc.dram_tensor("wk", [D, JG], BF16, kind="ExternalInput")
    wv_d = nc.dram_tensor("wv", [D, JG], BF16, kind="ExternalInput")
    wo_d = nc.dram_tensor("wo", [JG, D], BF16, kind="ExternalInput")
    cos_d = nc.dram_tensor("cosE", [128, N], BF16, kind="ExternalInput")
    sin_d = nc.dram_tensor("sinE", [128, N], BF16, kind="ExternalInput")
    rm_d = nc.dram_tensor("rmat", [128, 128], F32R, kind="ExternalInput")
    tm_d = nc.dram_tensor("trimask", [128, 128], F32, kind="ExternalInput")
    id_d = nc.dram_tensor("identb", [128, 128], BF16, kind="ExternalInput")
    y_d = nc.dram_tensor("yT", [D, N], F32, kind="ExternalOutput")

    xq_t = xq_d.ap().rearrange("(o p) s -> p o s", p=128)
    xk_t = xk_d.ap().rearrange("(o p) s -> p o s", p=128)
    wq_t = wq_d.ap().rearrange("(o p) j -> p o j", p=128)
    wk_t = wk_d.ap().rearrange("(o p) j -> p o j", p=128)
    wv_t = wv_d.ap().rearrange("(o p) j -> p o j", p=128)
    wo_t = wo_d.ap().rearrange("(o p) d -> p o d", p=128)

    with tile.TileContext(nc) as tc, ExitStack() as ctx:
        consts = ctx.enter_context(tc.tile_pool(name="consts", bufs=1))
        persist = ctx.enter_context(tc.tile_pool(name="persist", bufs=1))
        qt_pool = ctx.enter_context(tc.tile_pool(name="qt", bufs=2))
        y_pool = ctx.enter_context(tc.tile_pool(name="ysb", bufs=4))
        ictx = ctx.enter_context(ExitStack())
        x_pool = ictx.enter_context(tc.tile_pool(name="x", bufs=2))
        tmp_pool = ictx.enter_context(tc.tile_pool(name="tmp", bufs=2))
        es_pool = ictx.enter_context(tc.tile_pool(name="es", bufs=18))
        nr_pool = ictx.enter_context(tc.tile_pool(name="nr", bufs=2))
        ps_gen = ictx.enter_context(tc.tile_pool(name="psgen", bufs=2, space="PSUM"))
        ps_st = ictx.enter_context(tc.tile_pool(name="psst", bufs=2, space="PSUM"))
        ps_ov = ictx.enter_context(tc.tile_pool(name="psov", bufs=1, space="PSUM"))

        # ---- constants; order matters: first matmuls need wv + x(sc=0) ----
        wv_sb = consts.tile([128, KT8, JG], BF16, tag="wv")
        wq_sb = consts.tile([128, KT8, JG], BF16, tag="wq")
        wk_sb = consts.tile([128, KT8, JG], BF16, tag="wk")
        x0 = {}
        for nm in ("xk", "xq"):
            x0[nm] = x_pool.tile([128, KT8, SC], BF16, tag=nm, name=nm + "0")
        nc.sync.dma_start(out=x0["xk"][:, 0, :], in_=xk_t[:, 0, 0:SC])
        nc.sync.dma_start(out=wv_sb[:, 0, :], in_=wv_t[:, 0, :])
        nc.sync.dma_start(out=x0["xk"][:, 1:KT8, :], in_=xk_t[:, 1:KT8, 0:SC])
        nc.sync.dma_start(out=wv_sb[:, 1:KT8, :], in_=wv_t[:, 1:KT8, :])
        nc.sync.dma_start(out=x0["xq"][:, :, :], in_=xq_t[:, :, 0:SC])
        rmat = consts.tile([128, 128], F32R, tag="rmat")
        cosE = consts.tile([128, N], BF16, tag="cosE")
        sinE = consts.tile([128, N], BF16, tag="sinE")
        nc.sync.dma_start(out=wq_sb[:, :, 0:128], in_=wq_t[:, :, 0:128])
        nc.sync.dma_start(out=rmat[:, :], in_=rm_d[:, :])
        nc.sync.dma_start(out=cosE[:, :], in_=cos_d[:, :])
        nc.sync.dma_start(out=wq_sb[:, :, 128:256], in_=wq_t[:, :, 128:256])
        nc.sync.dma_start(out=sinE[:, :], in_=sin_d[:, :])
        for pg in range(2, 4):
            nc.sync.dma_start(out=wq_sb[:, :, pg * 128:(pg + 1) * 128],
                              in_=wq_t[:, :, pg * 128:(pg + 1) * 128])
        for pg in range(4):
            nc.sync.dma_start(out=wk_sb[:, :, pg * 128:(pg + 1) * 128],
                              in_=wk_t[:, :, pg * 128:(pg + 1) * 128])
        trimask = consts.tile([128, 128], F32, tag="trimask")
        nc.sync.dma_start(out=trimask[:, :], in_=tm_d[:, :])
        identb = consts.tile([128, 128], BF16, tag="identb")
        nc.sync.dma_start(out=identb[:, :], in_=id_d[:, :])
        wo_sb = consts.tile([128, 4, D], BF16, tag="wo")

        # persistent activations
        KTt = [[persist.tile([128, SC], BF16, tag=f"kt_{p}_{s}", name=f"kt_{p}_{s}")
                for s in range(NSC)] for p in range(NP)]
        Vt = [persist.tile([128, HG, HD + 1], BF16, tag=f"v_{i}", name=f"v_{i}")
              for i in range(NKB)]
        OTt = [[persist.tile([128, SC], BF16, tag=f"ot_{p}_{q}", name=f"ot_{p}_{q}")
                for q in range(NSC)] for p in range(NP)]

        def attention(p, qc, qts, fillers):
            h0, h1 = 2 * p, 2 * p + 1
            nkb = 4 * qc + 4
            qt = qts[p]
            # O accumulators: chain (qb, hl) at ov[:, qb*2+hl, 0:65]
            # (queries on partitions, head dim + denom on free; 128-col
            # pitch keeps each chain inside one PSUM bank)
            ov = ps_ov.tile([128, 8, 128], F32, tag="ov", name="ov")
            ess = [None] * nkb

            def s_exp(kb):
                diag = kb >= 4 * qc
                m = kb - 4 * qc
                skt = KTt[p][kb // 4]
                lo = (kb % 4) * KB
                c0 = m * KB if diag else 0
                st = ps_st.tile([128, 2, SC], F32, tag="st")
                es = es_pool.tile([128, 2, SC], BF16, tag="es")
                ess[kb] = (es, m if diag else 0)
                for hl in (0, 1):
                    r0, r1 = hl * 64, hl * 64 + 64
                    nc.tensor.matmul(
                        st[:, hl, c0:SC],
                        skt[r0:r1, lo:lo + KB],
                        qt[r0:r1, c0:SC],
                        start=True, stop=True)
                if diag:
                    # additive NEG mask on the diagonal block (pre-exp,
                    # on PSUM: keeps es single-writer)
                    nc.vector.tensor_add(
                        out=st[:, :, c0:c0 + KB],
                        in0=st[:, :, c0:c0 + KB],
                        in1=trimask[:, :].unsqueeze(1).broadcast_to([128, 2, KB]))
                nc.scalar.activation(
                    out=es[:, :, c0:SC], in_=st[:, :, c0:SC],
                    func=EXP, scale=float(HD) ** -0.5)

            # fillers go as LATE as possible: the ACT exp stream lags the
            # S matmuls by ~400ns/block, so PE needs independent work at
            # the tail of the kb loop, not the head
            nfil = len(fillers)
            fi = 0
            for kb in range(nkb):
                s_exp(kb)
                if nkb - kb <= nfil - fi:
                    fillers[fi]()
                    fi += 1

            # PV chain-major: HW PSUM allows only ONE open accumulation
            # group per bank, so walk one chain per bank at a time
            # (chains 0-3 live in bank A, 4-7 in bank B; the pair (g, g+4)
            # runs concurrently in different banks)
            def chain(c, kb):
                qb, hl = c // 2, c % 2
                if kb > 4 * qc + qb:
                    return False
                es, qb0 = ess[kb]
                if qb < qb0:
                    return False
                nc.tensor.matmul(
                    ov[:, c, 0:65],
                    es[:, hl, qb * KB:(qb + 1) * KB],
                    Vt[kb][:, (h0, h1)[hl], :],
                    start=(kb == 0), stop=(kb == 4 * qc + qb))
                return True

            for g in range(4):
                cA, cB = g, g + 4
                lenB = 4 * qc + cB // 2 + 1
                for kb in range(lenB):
                    chain(cA, kb)
                    chain(cB, kb)
                if fi < nfil:
                    fillers[fi]()
                    fi += 1
            while fi < nfil:
                fillers[fi]()
                fi += 1

            # softmax normalization: per-partition (query) reciprocal scale
            rc = nr_pool.tile([128, 8], F32, tag="rc")
            nc.vector.reciprocal(out=rc[:, :], in_=ov[:, :, 64])
            on = nr_pool.tile([128, 4, 128], BF16, tag="on")
            ov4 = ov[:, :, 0:64].rearrange("p (qb hl) c -> p qb hl c", qb=4)
            on4 = on[:, :, :].rearrange("p qb (hl c) -> p qb hl c", hl=2)
            rc4 = (rc[:, :].rearrange("p (qb hl) -> p qb hl", qb=4)
                   .unsqueeze(3).broadcast_to([128, 4, 2, 64]))
            nc.vector.tensor_mul(out=on4, in0=ov4, in1=rc4)

            def finish():
                # transpose O back to (head dim, query) layout for o_proj;
                # deferred into the next attention call so its PE work
                # doesn't wait on this pair's normalization chain
                genT = ps_gen.tile([128, SC], F32, tag="gen", name="onT")
                gb = genT[:, :].bitcast(BF16)
                for qb in range(4):
                    nc.tensor.transpose(gb[:, qb * 128:(qb + 1) * 128],
                                        on[:, qb, :], identb[:, :])
                nc.vector.tensor_copy(out=OTt[p][qc][:, :], in_=gb[:, 0:512])
            return finish

        def oproj_chunk(qc, dcs=None):
            for dc in (range(KT8) if dcs is None else dcs):
                yp = ps_gen.tile([128, SC], F32, tag="gen", name="yp")
                for kt in range(4):
                    nc.tensor.matmul(
                        yp[:, :],
                        wo_sb[:, kt, dc * 128:(dc + 1) * 128],
                        OTt[kt][qc][:, :],
                        start=(kt == 0), stop=(kt == 3))
                ysb = y_pool.tile([128, SC], F32, tag="ysb", name="ysb")
                if dc % 2 == 0:
                    nc.vector.tensor_copy(out=ysb[:, :], in_=yp[:, :])
                else:
                    nc.scalar.copy(out=ysb[:, :], in_=yp[:, :])
                nc.sync.dma_start(
                    out=y_d[dc * 128:(dc + 1) * 128, qc * SC:(qc + 1) * SC],
                    in_=ysb[:, :])

        def phase1_fillers(sc):
            """Closures emitting chunk sc's projections (x DMA, V proj,
            Q/K proj + RoPE), to be woven into the previous chunk's
            ACT-bound attention stream."""
            fill = []
            if sc == 0:
                xq_sb, xk_sb = x0["xq"], x0["xk"]
            else:
                xq_sb = x_pool.tile([128, KT8, SC], BF16, tag="xq", name="xq")
                xk_sb = x_pool.tile([128, KT8, SC], BF16, tag="xk", name="xk")

                def dma_x():
                    for k in range(KT8):
                        nc.sync.dma_start(out=xk_sb[:, k, :],
                                          in_=xk_t[:, k, sc * SC:(sc + 1) * SC])
                        nc.sync.dma_start(out=xq_sb[:, k, :],
                                          in_=xq_t[:, k, sc * SC:(sc + 1) * SC])
                    if sc == 1:
                        nc.sync.dma_start(out=wo_sb[:, :, :], in_=wo_t[:, :, :])
                fill.append(dma_x)

            qts = [qt_pool.tile([128, SC], BF16, tag=f"qt_{p}", name=f"qt_{p}")
                   for p in range(NP)]

            def vproj(ss):
                sidx = sc * 4 + ss
                vp = ps_gen.tile([128, SC], F32, tag="gen", name="vp")
                for k in range(KT8):
                    nc.tensor.matmul(
                        vp[:, :],
                        xk_sb[:, k, ss * 128:(ss + 1) * 128],
                        wv_sb[:, k, :],
                        start=(k == 0), stop=(k == KT8 - 1))
                vt = Vt[sidx]
                nc.vector.tensor_copy(
                    out=vt[:, :, 0:HD],
                    in_=vp[:, :].rearrange("p (h d) -> p h d", h=HG))
                nc.vector.memset(vt[:, :, HD:HD + 1], 1.0)

            stash = {}

            def qkproj_a(t, p):
                x_sb, w_sb = ((xq_sb, wq_sb), (xk_sb, wk_sb))[t]
                pp = ps_gen.tile([128, SC], F32, tag="gen", name="pp")
                for k in range(KT8):
                    nc.tensor.matmul(pp[:, :],
                                     w_sb[:, k, p * 128:(p + 1) * 128],
                                     x_sb[:, k, :],
                                     start=(k == 0), stop=(k == KT8 - 1))
                raw = tmp_pool.tile([128, SC], F32R, tag="raw")
                nc.vector.tensor_copy(out=raw[:, :], in_=pp[:, :])
                stash[(t, p)] = raw

            def qkproj_b(t, p):
                raw = stash.pop((t, p))
                rp = ps_gen.tile([128, SC], F32, tag="gen", name="rp")
                nc.tensor.matmul(rp[:, :], rmat[:, :], raw[:, :],
                                 start=True, stop=True)
                dest = qts[p] if t == 0 else KTt[p][sc]
                cs = slice(sc * SC, (sc + 1) * SC)
                nc.vector.tensor_mul(out=dest[:, :], in0=raw[:, :],
                                     in1=cosE[:, cs])
                tsin = tmp_pool.tile([128, SC], F32, tag="tsin")
                nc.vector.tensor_mul(out=tsin[:, :], in0=rp[:, :],
                                     in1=sinE[:, cs])
                nc.vector.tensor_add(out=dest[:, :], in0=dest[:, :],
                                     in1=tsin[:, :])

            # stage-interleaved order: the Pool copy of stage A gets >=2
            # filler slots of PE work before stage B's rmat needs it
            for ss in range(4):
                fill.append(lambda ss=ss: vproj(ss))
            tps = [(t, p) for t in range(2) for p in range(NP)]
            pend = []
            for tp in tps:
                fill.append(lambda tp=tp: qkproj_a(*tp))
                pend.append(tp)
                if len(pend) >= 2:
                    fill.append(lambda tp=pend.pop(0): qkproj_b(*tp))
            for tp in pend:
                fill.append(lambda tp=tp: qkproj_b(*tp))
            return qts, fill

        # ---- prologue: phase 1 for chunk 0 emitted inline ----
        qts_cur, fill0 = phase1_fillers(0)
        for f in fill0:
            f()

        pend_fin = None
        for sc in range(NSC):
            if sc + 1 < NSC:
                qts_nxt, fillers = phase1_fillers(sc + 1)
            else:
                qts_nxt, fillers = None, []
            per_p = [[] for _ in range(NP)]
            for p in range(NP):
                if sc > 0:
                    per_p[p].append(lambda p=p: oproj_chunk(sc - 1, [2 * p]))
                    per_p[p].append(
                        lambda p=p: oproj_chunk(sc - 1, [2 * p + 1]))
            # contiguous chunks keep staged fillers in emit order
            nf = len(fillers)
            cut = [round(nf * i / NP) for i in range(NP + 1)]
            for p in range(NP):
                per_p[p].extend(fillers[cut[p]:cut[p + 1]])
            for p in range(NP):
                if pend_fin is not None:
                    per_p[p].insert(0, pend_fin)
                pend_fin = attention(p, sc, qts_cur, per_p[p])
            qts_cur = qts_nxt
        pend_fin()

        # ---- final phase 3 chunk with fresh deep pools ----
        ictx.close()
        ps_y = ctx.enter_context(tc.tile_pool(name="psy", bufs=5, space="PSUM"))
        for dc in range(KT8):
            qc = NSC - 1
            yp = ps_y.tile([128, SC], F32, tag="yp", name="yp")
            for kt in range(4):
                nc.tensor.matmul(
                    yp[:, :],
                    wo_sb[:, kt, dc * 128:(dc + 1) * 128],
                    OTt[kt][qc][:, :],
                    start=(kt == 0), stop=(kt == 3))
            ysb = y_pool.tile([128, SC], F32, tag="ysb", name="ysb")
            if dc % 2 == 0:
                nc.vector.tensor_copy(out=ysb[:, :], in_=yp[:, :])
            else:
                nc.scalar.copy(out=ysb[:, :], in_=yp[:, :])
            nc.sync.dma_start(
                out=y_d[dc * 128:(dc + 1) * 128, qc * SC:(qc + 1) * SC],
                in_=ysb[:, :])

    nc.compile()
    return nc


def _host_consts(pos_enc):
    pe = np.asarray(pos_enc, np.float32)[0]          # (N, RD)
    cos = np.cos(pe).T                               # (RD, N)
    sin = np.sin(pe).T
    blk_c = np.ones((HD, N), np.float32)
    blk_c[:RD] = cos
    blk_s = np.zeros((HD, N), np.float32)
    blk_s[:RD] = sin
    cosE = np.tile(blk_c, (2, 1))                    # (128, N)
    sinE = np.tile(blk_s, (2, 1))
    rmat = np.zeros((128, 128), np.float32)
    for o in (0, HD):
        for i in range(RD // 2):
            rmat[o + 2 * i + 1, o + 2 * i] = -1.0
            rmat[o + 2 * i, o + 2 * i + 1] = 1.0
    r = np.arange(128)[:, None]
    c = np.arange(128)[None, :]
    trimask = np.where(c >= r, 0.0, NEG).astype(np.float32)
    identb = np.eye(128, dtype=np.float32)
    return cosE, sinE, rmat, trimask, identb


def kernel(x_q, x_kv, pos_enc, Wq, bq, Wk, bk, Wv, bv, Wo, bo, pad_mask):
    from concourse.bass_utils import run_bass_kernel_spmd

    if "nc" not in _CACHE:
        _CACHE["nc"] = _build_nc()
    nc = _CACHE["nc"]

    x_q = np.asarray(x_q, np.float32)
    x_kv = np.asarray(x_kv, np.float32)
    Wq = np.asarray(Wq, np.float32)
    Wk = np.asarray(Wk, np.float32)
    Wv = np.asarray(Wv, np.float32)
    Wo = np.asarray(Wo, np.float32)
    bo = np.asarray(bo, np.float32)

    cosE, sinE, rmat, trimask, identb = _host_consts(pos_enc)

    in_maps = []
    for core in range(8):
        b, g = core // 2, core % 2
        js = slice(g * JG, (g + 1) * JG)
        in_maps.append({
            "xqT": np.ascontiguousarray(x_q[b].T).astype(ml_dtypes.bfloat16),
            "xkT": np.ascontiguousarray(x_kv[b].T).astype(ml_dtypes.bfloat16),
            "wq": np.ascontiguousarray(Wq[:, js]).astype(ml_dtypes.bfloat16),
            "wk": np.ascontiguousarray(Wk[:, js]).astype(ml_dtypes.bfloat16),
            "wv": np.ascontiguousarray(Wv[:, js]).astype(ml_dtypes.bfloat16),
            "wo": np.ascontiguousarray(Wo[js, :]).astype(ml_dtypes.bfloat16),
            "cosE": cosE.astype(ml_dtypes.bfloat16), "sinE": sinE.astype(ml_dtypes.bfloat16),
            "rmat": rmat, "trimask": trimask,
            "identb": identb.astype(ml_dtypes.bfloat16),
        })

    res = run_bass_kernel_spmd(nc, in_maps, list(range(8)))

    out = np.empty((B, N, D), np.float32)
    for b in range(B):
        out[b] = res.results[2 * b]["yT"].T + res.results[2 * b + 1]["yT"].T
    out += bo
    return out

